# revision 12
# baseline (speedup 1.0000x reference)
"""Multi-head attention (B=2, S=2048, D=1024, H=16) on 8 Trainium2 cores.

Sharding: 2 heads per core (tensor-parallel on H). Each core computes its
2 heads' QKV projections, attention, and a partial output projection
(the 128 columns of the concat dim it owns); the host sums the 8 partial
outputs and adds the output bias.

Device dataflow per (batch, head):
  qT/kT = W x^T         [64, S]   (x^T supplied by host, bf16)
  vT    = Wv xv^T       -> PE-transpose -> v natural [S, 64] (+ones col)
  sT    = kT^T q        [t-block 128, s 512] transposed scores (psum)
  expS  = exp(sT/8)     (ScalarE, direct from psum)
  o~T/Z = [v|1]^T expS  [65, s]  (P@V with ones column -> row 64 = Z)
  oT    = o~T * (1/Z)   (recip + PE outer-product broadcast + DVE mul)
  y_c   = oT^T Wo_c^T   [s-block 128, 1024] partial output (psum->sbuf->DRAM)

All on-chip matmuls run in float32r (full PE rate at N=512) except the
projections, which take bf16 host inputs (halves HBM traffic).
"""

import os
import numpy as np
import ml_dtypes

B, S, D, H = 2, 2048, 1024, 16
HD = D // H          # 64
NCORES = 8
HPC = H // NCORES    # 2 heads per core
P = 128
SC = 512             # s-chunk width
NSC = S // SC        # 4
NKB = D // P         # 8 contraction blocks for projections
NTB = S // P         # 16 t-blocks

_BF16 = ml_dtypes.bfloat16

_nc_cache = {}
_runner_cache = {}


def build_nc(loop_k: int = 1):
    """Build (and cache) the per-core Bass module. loop_k>1 wraps the body
    in a hardware loop for timing measurements."""
    if loop_k in _nc_cache:
        return _nc_cache[loop_k]

    import concourse.bass as bass
    import concourse.mybir as mybir
    import concourse.tile as tile
    from concourse import bacc
    from concourse.masks import make_identity
    from contextlib import ExitStack

    f32 = mybir.dt.float32
    f32r = mybir.dt.float32r
    bf16 = mybir.dt.bfloat16
    AF = mybir.ActivationFunctionType

    nc = bacc.Bacc("TRN2", target_bir_lowering=False)

    xqT = nc.dram_tensor("xqT", [B, D, S], bf16, kind="ExternalInput")
    xkT = nc.dram_tensor("xkT", [B, D, S], bf16, kind="ExternalInput")
    xvT = nc.dram_tensor("xvT", [B, D, S], bf16, kind="ExternalInput")
    wq = nc.dram_tensor("wq", [D, P], bf16, kind="ExternalInput")
    wk = nc.dram_tensor("wk", [D, P], bf16, kind="ExternalInput")
    wv = nc.dram_tensor("wv", [D, P], bf16, kind="ExternalInput")
    bq = nc.dram_tensor("bq", [P, 1], f32, kind="ExternalInput")
    bk = nc.dram_tensor("bk", [P, 1], f32, kind="ExternalInput")
    bv = nc.dram_tensor("bv", [P, 1], f32, kind="ExternalInput")
    wo = nc.dram_tensor("wo", [P, D], f32, kind="ExternalInput")
    ypart = nc.dram_tensor("ypart", [B, S, D], f32, kind="ExternalOutput")

    with tile.TileContext(nc) as tc:
        with ExitStack() as ctx:
            const = ctx.enter_context(tc.tile_pool(name="const", bufs=1))
            xin = ctx.enter_context(tc.tile_pool(name="xin", bufs=12))
            qkv = ctx.enter_context(tc.tile_pool(name="qkv", bufs=2))
            vap = ctx.enter_context(tc.tile_pool(name="vap", bufs=2))
            expp = ctx.enter_context(tc.tile_pool(name="expp", bufs=4))
            smalls = ctx.enter_context(tc.tile_pool(name="smalls", bufs=4))
            yout = ctx.enter_context(tc.tile_pool(name="yout", bufs=3))
            # PSUM: "big" pool holds scores/proj/transpose/bc/y tiles
            # (slot = [128,1024] fp32 = 2 banks, 3 bufs = 6 banks);
            # "oacc" holds the two per-head P@V accumulators (2 banks).
            pp = ctx.enter_context(tc.tile_pool(name="pp", bufs=2, space="PSUM"))
            ppo = ctx.enter_context(tc.tile_pool(name="ppo", bufs=3, space="PSUM"))
            ppm = ctx.enter_context(tc.tile_pool(name="ppm", bufs=1, space="PSUM"))

            # ---- constants (outside the timing loop) ----
            wq_sb = const.tile([P, NKB, P], bf16, tag="wq")
            wk_sb = const.tile([P, NKB, P], bf16, tag="wk")
            wv_sb = const.tile([P, NKB, P], bf16, tag="wv")
            nc.sync.dma_start(wq_sb[:], wq.ap().rearrange("(a p) e -> p a e", p=P))
            nc.sync.dma_start(wk_sb[:], wk.ap().rearrange("(a p) e -> p a e", p=P))
            nc.sync.dma_start(wv_sb[:], wv.ap().rearrange("(a p) e -> p a e", p=P))
            wo_f32 = const.tile([P, D], f32, tag="wof")
            nc.sync.dma_start(wo_f32[:], wo[:, :])
            wo_sb = const.tile([P, D], f32r, tag="wo")
            nc.vector.tensor_copy(wo_sb[:], wo_f32[:])
            bq_sb = const.tile([P, 1], f32, tag="bq")
            bk_sb = const.tile([P, 1], f32, tag="bk")
            bv_sb = const.tile([P, 1], f32, tag="bv")
            nc.sync.dma_start(bq_sb[:], bq[:, :])
            nc.sync.dma_start(bk_sb[:], bk[:, :])
            nc.sync.dma_start(bv_sb[:], bv[:, :])
            ident_f32 = const.tile([P, P], f32, tag="identf")
            make_identity(nc, ident_f32[:])
            ident = const.tile([P, P], f32r, tag="ident")
            nc.vector.tensor_copy(ident[:], ident_f32[:])
            ones_f32 = const.tile([P, HD], f32, tag="onesf")
            nc.vector.memset(ones_f32[:], 1.0)
            ones_row = const.tile([1, HD], f32r, tag="ones")
            nc.vector.tensor_copy(ones_row[:], ones_f32[0:1, 0:HD])

            def r(ap):
                return ap

            def body():
                for b in range(B):
                    # ---------- projections ----------
                    qT_sb = qkv.tile([P, S], f32r, tag="qT")
                    kT_sb = qkv.tile([P, S], f32r, tag="kT")
                    vT_sb = qkv.tile([P, S], f32r, tag="vT")
                    for (w_sb, xdram, b_sb, dest) in (
                        (wk_sb, xkT, bk_sb, kT_sb),
                        (wv_sb, xvT, bv_sb, vT_sb),
                        (wq_sb, xqT, bq_sb, qT_sb),
                    ):
                        # full-row k-block tiles, reused by all 4 s-chunks
                        xts = []
                        for kb in range(NKB):
                            xt = xin.tile([P, S], bf16, tag="xt")
                            nc.sync.dma_start(
                                xt[:], xdram[b, kb * P:(kb + 1) * P, :]
                            )
                            xts.append(xt)
                        for sc in range(NSC):
                            ps = ppm.tile([P, SC], f32, tag="m")
                            for kb in range(NKB):
                                nc.tensor.matmul(
                                    ps[:], w_sb[:, kb, :],
                                    xts[kb][:, sc * SC:(sc + 1) * SC],
                                    start=(kb == 0), stop=(kb == NKB - 1),
                                )
                            nc.vector.tensor_scalar_add(
                                dest[:, sc * SC:(sc + 1) * SC], ps[:], b_sb[:]
                            )

                    # ---------- v natural (+ones) via PE transpose ----------
                    v_aug = vap.tile([P, HPC, NTB, HD + 1], f32r, tag="vaug")
                    nc.vector.tensor_copy(
                        v_aug[:, :, :, HD], ones_f32[:, 0:HPC * NTB]
                    )
                    for tb in range(NTB):
                        pst = ppm.tile([P, P], f32r, tag="m")
                        nc.tensor.transpose(
                            r(pst[:]), r(vT_sb[:, tb * P:(tb + 1) * P]), r(ident[:])
                        )
                        nc.vector.tensor_copy(v_aug[:, 0, tb, 0:HD], pst[:, 0:HD])
                        nc.vector.tensor_copy(v_aug[:, 1, tb, 0:HD], pst[:, HD:P])

                    # ---------- attention ----------
                    oT_sb = qkv.tile([P, S], f32r, tag="oT")
                    for sc in range(NSC):
                        o_h0 = ppo.tile([HD + 1, SC], f32, tag="oacc")
                        o_h1 = ppo.tile([HD + 1, SC], f32, tag="oacc")
                        for tb in range(NTB):
                            ssl = slice(sc * SC, (sc + 1) * SC)
                            tsl = slice(tb * P, (tb + 1) * P)
                            ps_sc = pp.tile([P, 2 * SC], f32, tag="sc")
                            nc.tensor.matmul(
                                ps_sc[:, 0:SC], r(kT_sb[0:HD, tsl]), r(qT_sb[0:HD, ssl]),
                                start=True, stop=True, tile_position=(0, 0),
                            )
                            nc.tensor.matmul(
                                ps_sc[:, SC:2 * SC], r(kT_sb[HD:P, tsl]), r(qT_sb[HD:P, ssl]),
                                start=True, stop=True, tile_position=(64, 0),
                            )
                            ex = expp.tile([P, 2 * SC], f32r, tag="ex")
                            nc.scalar.activation(ex[:], ps_sc[:], AF.Exp, scale=0.125)
                            nc.tensor.matmul(
                                o_h0[:], r(v_aug[:, 0, tb, :]), r(ex[:, 0:SC]),
                                start=(tb == 0), stop=(tb == NTB - 1),
                            )
                            nc.tensor.matmul(
                                o_h1[:], r(v_aug[:, 1, tb, :]), r(ex[:, SC:2 * SC]),
                                start=(tb == 0), stop=(tb == NTB - 1),
                            )
                        # normalize both heads for this s-chunk
                        for h, o_ps in ((0, o_h0), (1, o_h1)):
                            rz = smalls.tile([1, SC], f32r, tag="rz")
                            with nc.allow_low_precision(
                                reason="fp32r rounding for PE broadcast"
                            ):
                                nc.vector.reciprocal(rz[:], o_ps[HD:HD + 1, :])
                            bc = ppo.tile([HD, SC], f32, tag="oacc")
                            nc.tensor.matmul(
                                bc[:], r(ones_row[:]), r(rz[:]), start=True, stop=True
                            )
                            bc_sb = smalls.tile([HD, SC], f32, tag="bcs")
                            nc.vector.tensor_copy(bc_sb[:], bc[:])
                            nc.vector.tensor_mul(
                                oT_sb[h * HD:(h + 1) * HD, sc * SC:(sc + 1) * SC],
                                o_ps[0:HD, :], bc_sb[:],
                            )

                        # ---- output projection for this s-chunk's 4 blocks ----
                        for sb in range(4 * sc, 4 * sc + 4):
                            osl = r(oT_sb[:, sb * P:(sb + 1) * P])
                            ysb = yout.tile([P, D], f32, tag="y")
                            for half in range(2):
                                psy = ppo.tile([P, SC], f32, tag="oacc")
                                nc.tensor.matmul(
                                    psy[:], osl, r(wo_sb[:, half * SC:(half + 1) * SC]),
                                    start=True, stop=True,
                                )
                                nc.vector.tensor_copy(
                                    ysb[:, half * SC:(half + 1) * SC], psy[:]
                                )
                            nc.sync.dma_start(
                                ypart[b, sb * P:(sb + 1) * P, :], ysb[:]
                            )

            if loop_k == 1:
                body()
            else:
                with tc.For_i(
                    0, loop_k, 1,
                    hint_engines=(
                        mybir.EngineType.PE,
                        mybir.EngineType.DVE,
                        mybir.EngineType.Activation,
                        mybir.EngineType.SP,
                        mybir.EngineType.Pool,
                    ),
                ):
                    body()

    nc.compile()
    _nc_cache[loop_k] = nc
    return nc


def make_in_maps(inputs):
    """Host-side sharding: transpose activations to [B, D, S] bf16, slice
    per-head weights per core."""
    query, key, value = inputs["query"], inputs["key"], inputs["value"]
    Wq, bq, Wk, bk, Wv, bv = (
        inputs["Wq"], inputs["bq"], inputs["Wk"], inputs["bk"],
        inputs["Wv"], inputs["bv"],
    )
    Wo, bo = inputs["Wo"], inputs["bo"]

    xqT = np.ascontiguousarray(np.transpose(query, (0, 2, 1))).astype(_BF16)
    xkT = np.ascontiguousarray(np.transpose(key, (0, 2, 1))).astype(_BF16)
    xvT = np.ascontiguousarray(np.transpose(value, (0, 2, 1))).astype(_BF16)

    in_maps = []
    for c in range(NCORES):
        hs = slice(c * HPC, (c + 1) * HPC)
        # [HPC, HD, D] -> [D, HPC*HD]
        wq_c = np.ascontiguousarray(
            Wq[hs].reshape(HPC * HD, D).T).astype(_BF16)
        wk_c = np.ascontiguousarray(
            Wk[hs].reshape(HPC * HD, D).T).astype(_BF16)
        wv_c = np.ascontiguousarray(
            Wv[hs].reshape(HPC * HD, D).T).astype(_BF16)
        bq_c = np.ascontiguousarray(bq[hs].reshape(P, 1)).astype(np.float32)
        bk_c = np.ascontiguousarray(bk[hs].reshape(P, 1)).astype(np.float32)
        bv_c = np.ascontiguousarray(bv[hs].reshape(P, 1)).astype(np.float32)
        wo_c = np.ascontiguousarray(Wo[:, c * P:(c + 1) * P].T).astype(np.float32)
        in_maps.append({
            "xqT": xqT, "xkT": xkT, "xvT": xvT,
            "wq": wq_c, "wk": wk_c, "wv": wv_c,
            "bq": bq_c, "bk": bk_c, "bv": bv_c,
            "wo": wo_c,
        })
    return in_maps


def make_runner(nc, n_cores=NCORES):
    """Cached jitted shard_map runner (mirrors bass2jax.run_bass_via_pjrt
    without donation so it can be re-invoked for timing)."""
    key = id(nc)
    if key in _runner_cache:
        return _runner_cache[key]
    import jax
    from jax.sharding import Mesh, PartitionSpec
    from jax.experimental.shard_map import shard_map
    import concourse.mybir as mybir
    from concourse import bass2jax

    bass2jax.install_neuronx_cc_hook()
    partition_name = nc.partition_id_tensor.name if nc.partition_id_tensor else None
    in_names, out_names, out_avals = [], [], []
    for alloc in nc.m.functions[0].allocations:
        if not isinstance(alloc, mybir.MemoryLocationSet):
            continue
        name = alloc.memorylocations[0].name
        if alloc.kind == "ExternalInput":
            if name != partition_name:
                in_names.append(name)
        elif alloc.kind == "ExternalOutput":
            out_names.append(name)
            out_avals.append(
                jax.core.ShapedArray(
                    tuple(alloc.tensor_shape), mybir.dt.np(alloc.dtype))
            )
    all_in_names = list(in_names) + ([partition_name] if partition_name else [])

    def _body(*args):
        operands = list(args)
        if partition_name is not None:
            operands.append(bass2jax.partition_id_tensor())
        outs = bass2jax._bass_exec_p.bind(
            *operands, out_avals=tuple(out_avals),
            in_names=tuple(all_in_names), out_names=tuple(out_names),
            lowering_input_output_aliases=(),
            sim_require_finite=False, sim_require_nnan=False, nc=nc)
        return tuple(outs)

    devices = jax.devices()[:n_cores]
    mesh = Mesh(np.asarray(devices), ("core",))
    fn = jax.jit(shard_map(
        _body, mesh=mesh,
        in_specs=(PartitionSpec("core"),) * len(in_names),
        out_specs=(PartitionSpec("core"),) * len(out_names),
        check_rep=False))
    out = (fn, in_names, out_names, out_avals)
    _runner_cache[key] = out
    return out


def run_on_cores(nc, in_maps):
    """Run the module on the 8 cores; returns list of per-core out dicts."""
    import jax
    fn, in_names, out_names, out_avals = make_runner(nc)
    concat_in = [
        np.concatenate([m[nm] for m in in_maps], axis=0) for nm in in_names
    ]
    outs = jax.block_until_ready(fn(*concat_in))
    res = []
    for c in range(len(in_maps)):
        d = {}
        for i, nm in enumerate(out_names):
            shp = out_avals[i].shape
            d[nm] = np.asarray(outs[i]).reshape(len(in_maps), *shp)[c]
        res.append(d)
    return res


def postprocess(results, inputs):
    """Sum per-core partial outputs and add the output bias."""
    acc = np.zeros((B, S, D), dtype=np.float64)
    for r in results:
        acc += r["ypart"].astype(np.float64)
    acc += inputs["bo"].astype(np.float64)
    return acc.astype(np.float32)


def kernel(**inputs) -> np.ndarray:
    inputs = {k: np.asarray(v) for k, v in inputs.items()}
    nc = build_nc(loop_k=1)
    in_maps = make_in_maps(inputs)
    results = run_on_cores(nc, in_maps)
    return postprocess(results, inputs)
